# revision 6
# baseline (speedup 1.0000x reference)
"""DifferentiableMatcher Trainium2 kernel (v2).

cost[k, n] = 1 - <pred_k, gt_n> over HW=512*512, then 5 Sinkhorn iterations
(row/col logsumexp normalizations) and exp.

Strategy (8 NeuronCores):
  - Shard the HW contraction: core c owns HW slice [c*32768, (c+1)*32768).
  - Inputs cast to fp16 on host (halves HBM traffic; fp32 PSUM accumulate).
  - Host packs each shard so SBUF partition p holds runs of FB=4 HW elements
    per (q, k); DMA per partition is fully contiguous.
  - Per core: 256 accumulating fp16 matmuls -> partial dot [100, 50] in PSUM.
    6 blocks of 32 chunk-pairs + 4 tail blocks of 16 keep the DMA dense
    while shrinking the PE drain after the last block.
  - AllReduce (20KB fp32) across the 8 cores, then Sinkhorn runs replicated
    in fp32 log space exactly like the reference (max-subtracted logsumexp).

v2 changes vs v1:
  - ACT exp/ln table preloaded via a dummy activation at kernel start
    (removes the 1.3us ACT_TABLE_LOAD from the post-AllReduce critical path).
  - Final Sinkhorn col step's normalize+exp fused into one activation
    (bias = -max - ln(sum), per-partition AP).
"""

import numpy as np

K = 100
N = 50
HW = 512 * 512
CORES = 8
SHARD = HW // CORES  # 32768
P = 128
FB = 4
Q = SHARD // (P * FB)  # 64 q-steps per core
# 6 big blocks (8 q-steps) + 4 small tail blocks (4 q-steps) = 64 q-steps;
# smaller tail blocks cut the post-DMA PE drain roughly in half.
NBLKA = 6
QBA = 8
NBLKB = 4
QBB = 4
TEMP = 0.1
ITERS = 5

_CACHE = {}

TRACE = False
TRACE_KW = {}
LAST_RESULT = None


def _patch_act_tables():
    """Make the combined Exp+Ln table set the only candidate for Exp/Ln so
    the compiler emits one table load instead of thrashing per activation.
    Set positions (= act_func_set_ids) are preserved."""
    import concourse.hw_specs as hw_specs
    from concourse import bacc as bacc_mod
    from concourse import mybir

    if getattr(bacc_mod, "_act_tables_patched", False):
        return
    orig = hw_specs.get_activation_tables

    def patched(arch):
        t = orig(arch)
        exp = mybir.ActivationFunctionType.Exp
        ln = mybir.ActivationFunctionType.Ln
        out = {}
        for name, funcs in t.items():
            if (exp in funcs) != (ln in funcs):
                funcs = funcs - {exp, ln}
            out[name] = funcs
        return out

    bacc_mod.get_activation_tables = patched
    bacc_mod._act_tables_patched = True


def _build():
    from concourse import bacc, tile, mybir
    from concourse.masks import make_identity

    _patch_act_tables()

    f16 = mybir.dt.float16
    f32 = mybir.dt.float32
    nc = bacc.Bacc("TRN2", target_bir_lowering=False, debug=False, enable_asserts=False, num_devices=CORES, monotonic_sem_count=0, enable_partition_id=False)
    p_in = nc.dram_tensor(
        "p_in", [NBLKA, P, QBA * K * FB], f16, kind="ExternalInput"
    ).ap()
    g_in = nc.dram_tensor(
        "g_in", [NBLKA, P, QBA * N * FB], f16, kind="ExternalInput"
    ).ap()
    p_in2 = nc.dram_tensor(
        "p_in2", [NBLKB, P, QBB * K * FB], f16, kind="ExternalInput"
    ).ap()
    g_in2 = nc.dram_tensor(
        "g_in2", [NBLKB, P, QBB * N * FB], f16, kind="ExternalInput"
    ).ap()
    out = nc.dram_tensor("out", [N, K], f32, kind="ExternalOutput").ap()

    Exp = mybir.ActivationFunctionType.Exp
    Ln = mybir.ActivationFunctionType.Ln

    with tile.TileContext(nc) as tc:
        with (
            tc.tile_pool(name="pp", bufs=NBLKA + NBLKB) as pp,
            tc.tile_pool(name="gp", bufs=NBLKA + NBLKB) as gp,
            tc.tile_pool(name="sk", bufs=1) as sk,
            tc.tile_pool(name="cps", bufs=1, space="PSUM") as cps,
            tc.tile_pool(name="tps", bufs=2, space="PSUM") as tps,
            tc.tile_pool(name="dram", bufs=1, space="DRAM") as dram,
        ):
            # Force the natural_log_exp table set to load now, while the
            # DMAs stream; the Sinkhorn tail then starts without the
            # 1.3us ACT_TABLE_LOAD on its critical path.
            warm = sk.tile([1, 1], f32)
            nc.vector.memset(warm, 0.0)
            nc.scalar.activation(out=warm, in_=warm, func=Exp)

            ident = sk.tile([P, P], f32)
            make_identity(nc, ident)

            C = cps.tile([K, N], f32)
            blocks = [(p_in, g_in, b, QBA) for b in range(NBLKA)] + [
                (p_in2, g_in2, b, QBB) for b in range(NBLKB)
            ]
            for bi, (psrc, gsrc, b, qb) in enumerate(blocks):
                PT = pp.tile([P, qb * K * FB], f16)
                GT = gp.tile([P, qb * N * FB], f16)
                if bi % 2 == 0:
                    nc.scalar.dma_start(out=GT, in_=gsrc[b])
                    nc.sync.dma_start(out=PT, in_=psrc[b])
                else:
                    nc.sync.dma_start(out=GT, in_=gsrc[b])
                    nc.scalar.dma_start(out=PT, in_=psrc[b])
                PT4 = PT.rearrange("p (q k f) -> p q k f", k=K, f=FB)
                GT4 = GT.rearrange("p (q n f) -> p q n f", n=N, f=FB)
                for q in range(qb):
                    for f in range(FB):
                        nc.tensor.matmul(
                            C,
                            PT4[:, q, :, f],
                            GT4[:, q, :, f],
                            start=(bi == 0 and q == 0 and f == 0),
                            stop=(
                                bi == len(blocks) - 1
                                and q == qb - 1
                                and f == FB - 1
                            ),
                        )

            # partial dot [K,N] -> (scale 1/TEMP) -> SBUF -> DRAM ->
            # AllReduce -> SBUF
            c_sb = sk.tile([K, N], f32)
            nc.vector.tensor_scalar(
                out=c_sb, in0=C, scalar1=1.0 / TEMP, scalar2=None,
                op0=mybir.AluOpType.mult,
            )
            din = dram.tile([K, N], f32)
            dout = dram.tile([K, N], f32, addr_space="Shared")
            nc.sync.dma_start(out=din, in_=c_sb)
            nc.gpsimd.collective_compute(
                "AllReduce",
                mybir.AluOpType.add,
                replica_groups=[list(range(CORES))],
                ins=[din.opt()],
                outs=[dout.opt()],
            )
            csum = sk.tile([K, N], f32)
            nc.sync.dma_start(out=csum, in_=dout)

            # log_alpha = (dot - 1)/TEMP = csum - 1/TEMP up to an additive
            # constant which cancels in the first row logsumexp.
            cur = csum  # [K, N]
            for it in range(ITERS):
                # rows: lse over free dim of [K, N]
                nM = sk.tile([K, 1], f32)
                nc.vector.reduce_max(
                    out=nM, in_=cur, axis=mybir.AxisListType.X, negate=True
                )
                E = tps.tile([K, N], f32, tag="escr", bufs=1)
                S = sk.tile([K, 1], f32)
                nc.scalar.activation(out=E, in_=cur, func=Exp, bias=nM, accum_out=S)
                lS = sk.tile([K, 1], f32)
                nc.scalar.activation(out=lS, in_=S, func=Ln)
                L2 = sk.tile([K, N], f32)
                nc.vector.tensor_scalar(
                    out=L2,
                    in0=cur,
                    scalar1=nM,
                    scalar2=lS,
                    op0=mybir.AluOpType.add,
                    op1=mybir.AluOpType.subtract,
                )
                # cols: transpose, lse over free
                TpP = tps.tile([N, K], f32)
                nc.tensor.transpose(TpP, L2, ident[:K, :K])
                nM2 = sk.tile([N, 1], f32)
                nc.vector.reduce_max(
                    out=nM2, in_=TpP, axis=mybir.AxisListType.X, negate=True
                )
                E2 = tps.tile([N, K], f32, tag="escr2", bufs=1)
                S2 = sk.tile([N, 1], f32)
                nc.scalar.activation(out=E2, in_=TpP, func=Exp, bias=nM2, accum_out=S2)
                lS2 = sk.tile([N, 1], f32)
                nc.scalar.activation(out=lS2, in_=S2, func=Ln)
                if it < ITERS - 1:
                    Lt2 = sk.tile([N, K], f32)
                    nc.vector.tensor_scalar(
                        out=Lt2,
                        in0=TpP,
                        scalar1=nM2,
                        scalar2=lS2,
                        op0=mybir.AluOpType.add,
                        op1=mybir.AluOpType.subtract,
                    )
                    Tp2P = tps.tile([K, N], f32)
                    nc.tensor.transpose(Tp2P, Lt2, ident[:N, :N])
                    cur = Tp2P
                else:
                    # final col step: fuse normalize+exp into one activation
                    # res = exp(TpP + nM2 - lS2); host transposes [N,K]->[K,N]
                    bias_t = sk.tile([N, 1], f32)
                    nc.vector.tensor_scalar(
                        out=bias_t, in0=nM2, scalar1=lS2, scalar2=None,
                        op0=mybir.AluOpType.subtract,
                    )
                    res = sk.tile([N, K], f32)
                    nc.scalar.activation(out=res, in_=TpP, func=Exp, bias=bias_t)
                    nc.sync.dma_start(out=out, in_=res)

    nc.compile()
    return nc


def _get_nc():
    if "nc" not in _CACHE:
        _CACHE["nc"] = _build()
    return _CACHE["nc"]


def _get_runner():
    """Cached PJRT executable (mirrors bass2jax.run_bass_via_pjrt's multi-core
    branch) so repeat kernel() calls skip retracing/recompiling."""
    if "runner" in _CACHE:
        return _CACHE["runner"]
    import jax
    from jax.experimental.shard_map import shard_map
    from jax.sharding import Mesh, PartitionSpec

    from concourse import bass2jax, mybir

    nc = _get_nc()
    bass2jax.install_neuronx_cc_hook()
    assert nc.dbg_addr is None
    partition_name = nc.partition_id_tensor.name if nc.partition_id_tensor else None

    in_names, out_names, out_avals, out_shapes = [], [], [], []
    for alloc in nc.m.functions[0].allocations:
        if not isinstance(alloc, mybir.MemoryLocationSet):
            continue
        name = alloc.memorylocations[0].name
        if alloc.kind == "ExternalInput":
            if name != partition_name:
                in_names.append(name)
        elif alloc.kind == "ExternalOutput":
            shape = tuple(alloc.tensor_shape)
            dtype = mybir.dt.np(alloc.dtype)
            out_avals.append(jax.core.ShapedArray(shape, dtype))
            out_shapes.append((name, shape, dtype))
            out_names.append(name)
    n_params = len(in_names)
    n_outs = len(out_names)
    all_in_names = list(in_names) + list(out_names)
    if partition_name is not None:
        all_in_names.append(partition_name)
    donate = tuple(range(n_params, n_params + n_outs))

    def _body(*args):
        operands = list(args)
        if partition_name is not None:
            operands.append(bass2jax.partition_id_tensor())
        outs = bass2jax._bass_exec_p.bind(
            *operands,
            out_avals=tuple(out_avals),
            in_names=tuple(all_in_names),
            out_names=tuple(out_names),
            lowering_input_output_aliases=(),
            sim_require_finite=True,
            sim_require_nnan=True,
            nc=nc,
        )
        return tuple(outs)

    devices = jax.devices()[:CORES]
    mesh = Mesh(np.asarray(devices), ("core",))
    in_specs = (PartitionSpec("core"),) * (n_params + n_outs)
    out_specs = (PartitionSpec("core"),) * n_outs
    sharded = jax.jit(
        shard_map(
            _body, mesh=mesh, in_specs=in_specs, out_specs=out_specs, check_rep=False
        ),
        donate_argnums=donate,
        keep_unused=True,
    )
    _CACHE["runner"] = (sharded, in_names, out_shapes)
    return _CACHE["runner"]


def _pack(arr, rows):
    # arr [rows, HW] fp32 -> fp16 packed per core: q-step q of core c covers
    # HW [c*SHARD + q*512, +512), SBUF partition p holds FB=4 consecutive
    # elements per (q, row).  First NBLKA*QBA q-steps go to the big blocks,
    # the rest to the small tail blocks.
    v = arr.reshape(rows, CORES, Q, P, FB).transpose(1, 2, 3, 0, 4)
    v = v.astype(np.float16)  # [CORES, Q, P, rows, FB]
    na = NBLKA * QBA
    va = v[:, :na].reshape(CORES, NBLKA, QBA, P, rows, FB).transpose(0, 1, 3, 2, 4, 5)
    vb = v[:, na:].reshape(CORES, NBLKB, QBB, P, rows, FB).transpose(0, 1, 3, 2, 4, 5)
    return (
        np.ascontiguousarray(va).reshape(CORES, NBLKA, P, QBA * rows * FB),
        np.ascontiguousarray(vb).reshape(CORES, NBLKB, P, QBB * rows * FB),
    )


def kernel(pred_masks, gt_masks):
    global LAST_RESULT
    from concourse import bass_utils

    pred = np.ascontiguousarray(np.asarray(pred_masks, dtype=np.float32)).reshape(
        K, HW
    )
    gt = np.ascontiguousarray(np.asarray(gt_masks, dtype=np.float32)).reshape(N, HW)
    pka, pkb = _pack(pred, K)
    gka, gkb = _pack(gt, N)
    in_maps = [
        {"p_in": pka[c], "g_in": gka[c], "p_in2": pkb[c], "g_in2": gkb[c]}
        for c in range(CORES)
    ]
    if TRACE:
        nc = _get_nc()
        res = bass_utils.run_bass_kernel_spmd(
            nc, in_maps, core_ids=list(range(CORES)), trace=TRACE, **TRACE_KW
        )
        LAST_RESULT = res
        o = np.asarray(res.results[0]["out"], dtype=np.float32)
        return np.ascontiguousarray(o.T).reshape(1, K, N)

    sharded, in_names, out_shapes = _get_runner()
    concat_in = [
        np.concatenate([in_maps[c][name] for c in range(CORES)], axis=0)
        for name in in_names
    ]
    concat_zeros = [
        np.zeros((CORES * shape[0], *shape[1:]), dtype) for _, shape, dtype in out_shapes
    ]
    out_arrs = sharded(*concat_in, *concat_zeros)
    out0 = np.asarray(out_arrs[0]).reshape(CORES, N, K)[0]
    return np.ascontiguousarray(out0.astype(np.float32).T).reshape(1, K, N)


# revision 7
# speedup vs baseline: 1.0888x; 1.0888x over previous
"""DifferentiableMatcher Trainium2 kernel (v2).

cost[k, n] = 1 - <pred_k, gt_n> over HW=512*512, then 5 Sinkhorn iterations
(row/col logsumexp normalizations) and exp.

Strategy (8 NeuronCores):
  - Shard the HW contraction: core c owns HW slice [c*32768, (c+1)*32768).
  - Inputs cast to fp16 on host (halves HBM traffic; fp32 PSUM accumulate).
  - Host packs each shard so SBUF partition p holds runs of FB=4 HW elements
    per (q, k); DMA per partition is fully contiguous.
  - Per core: 256 accumulating fp16 matmuls -> partial dot [100, 50] in PSUM.
    6 blocks of 32 chunk-pairs + 4 tail blocks of 16 keep the DMA dense
    while shrinking the PE drain after the last block.
  - AllReduce (20KB fp32) across the 8 cores, then Sinkhorn runs replicated
    in fp32 log space exactly like the reference (max-subtracted logsumexp).

v2 changes vs v1:
  - ACT exp/ln table preloaded via a dummy activation at kernel start
    (removes the 1.3us ACT_TABLE_LOAD from the post-AllReduce critical path).
  - Final Sinkhorn col step's normalize+exp fused into one activation
    (bias = -max - ln(sum), per-partition AP).
"""

import numpy as np

K = 100
N = 50
HW = 512 * 512
CORES = 8
SHARD = HW // CORES  # 32768
P = 128
FB = 4
Q = SHARD // (P * FB)  # 64 q-steps per core
# 6 big blocks (8 q-steps) + 4 small tail blocks (4 q-steps) = 64 q-steps;
# smaller tail blocks cut the post-DMA PE drain roughly in half.
NBLKA = 6
QBA = 8
NBLKB = 4
QBB = 4
TEMP = 0.1
ITERS = 5

_CACHE = {}

TRACE = False
TRACE_KW = {}
LAST_RESULT = None


def _patch_act_tables():
    """Make the combined Exp+Ln table set the only candidate for Exp/Ln so
    the compiler emits one table load instead of thrashing per activation.
    Set positions (= act_func_set_ids) are preserved."""
    import concourse.hw_specs as hw_specs
    from concourse import bacc as bacc_mod
    from concourse import mybir

    if getattr(bacc_mod, "_act_tables_patched", False):
        return
    orig = hw_specs.get_activation_tables

    def patched(arch):
        t = orig(arch)
        exp = mybir.ActivationFunctionType.Exp
        ln = mybir.ActivationFunctionType.Ln
        out = {}
        for name, funcs in t.items():
            if (exp in funcs) != (ln in funcs):
                funcs = funcs - {exp, ln}
            out[name] = funcs
        return out

    bacc_mod.get_activation_tables = patched
    bacc_mod._act_tables_patched = True


def _build():
    from concourse import bacc, tile, mybir
    from concourse.masks import make_identity

    _patch_act_tables()

    f16 = mybir.dt.float16
    f32 = mybir.dt.float32
    nc = bacc.Bacc("TRN2", target_bir_lowering=False, debug=False, enable_asserts=False, num_devices=CORES, monotonic_sem_count=0, enable_partition_id=False)
    p_in = nc.dram_tensor(
        "p_in", [NBLKA, P, QBA * K * FB], f16, kind="ExternalInput"
    ).ap()
    g_in = nc.dram_tensor(
        "g_in", [NBLKA, P, QBA * N * FB], f16, kind="ExternalInput"
    ).ap()
    p_in2 = nc.dram_tensor(
        "p_in2", [NBLKB, P, QBB * K * FB], f16, kind="ExternalInput"
    ).ap()
    g_in2 = nc.dram_tensor(
        "g_in2", [NBLKB, P, QBB * N * FB], f16, kind="ExternalInput"
    ).ap()
    out = nc.dram_tensor("out", [N, K], f32, kind="ExternalOutput").ap()

    Exp = mybir.ActivationFunctionType.Exp
    Ln = mybir.ActivationFunctionType.Ln

    with tile.TileContext(nc) as tc:
        with (
            tc.tile_pool(name="pp", bufs=NBLKA + NBLKB) as pp,
            tc.tile_pool(name="gp", bufs=NBLKA + NBLKB) as gp,
            tc.tile_pool(name="sk", bufs=1) as sk,
            tc.tile_pool(name="cps", bufs=1, space="PSUM") as cps,
            tc.tile_pool(name="tps", bufs=2, space="PSUM") as tps,
            tc.tile_pool(name="dram", bufs=1, space="DRAM") as dram,
        ):
            # Force the natural_log_exp table set to load now, while the
            # DMAs stream; the Sinkhorn tail then starts without the
            # 1.3us ACT_TABLE_LOAD on its critical path.
            warm = sk.tile([1, 1], f32)
            nc.vector.memset(warm, 0.0)
            nc.scalar.activation(out=warm, in_=warm, func=Exp)

            ident = sk.tile([P, P], f32)
            make_identity(nc, ident)

            C = cps.tile([K, N], f32)
            blocks = [(p_in, g_in, b, QBA) for b in range(NBLKA)] + [
                (p_in2, g_in2, b, QBB) for b in range(NBLKB)
            ]
            for bi, (psrc, gsrc, b, qb) in enumerate(blocks):
                PT = pp.tile([P, qb * K * FB], f16)
                GT = gp.tile([P, qb * N * FB], f16)
                if bi % 2 == 0:
                    nc.scalar.dma_start(out=GT, in_=gsrc[b])
                    nc.sync.dma_start(out=PT, in_=psrc[b])
                else:
                    nc.sync.dma_start(out=GT, in_=gsrc[b])
                    nc.scalar.dma_start(out=PT, in_=psrc[b])
                PT4 = PT.rearrange("p (q k f) -> p q k f", k=K, f=FB)
                GT4 = GT.rearrange("p (q n f) -> p q n f", n=N, f=FB)
                for q in range(qb):
                    for f in range(FB):
                        nc.tensor.matmul(
                            C,
                            PT4[:, q, :, f],
                            GT4[:, q, :, f],
                            start=(bi == 0 and q == 0 and f == 0),
                            stop=(
                                bi == len(blocks) - 1
                                and q == qb - 1
                                and f == FB - 1
                            ),
                        )

            # partial dot [K,N] -> (scale 1/TEMP) -> SBUF -> DRAM ->
            # AllReduce -> SBUF
            c_sb = sk.tile([K, N], f32)
            nc.vector.tensor_scalar(
                out=c_sb, in0=C, scalar1=1.0 / TEMP, scalar2=None,
                op0=mybir.AluOpType.mult,
            )
            din = dram.tile([K, N], f32)
            dout = dram.tile([K, N], f32, addr_space="Shared")
            nc.sync.dma_start(out=din, in_=c_sb)
            nc.gpsimd.collective_compute(
                "AllReduce",
                mybir.AluOpType.add,
                replica_groups=[list(range(CORES))],
                ins=[din.opt()],
                outs=[dout.opt()],
            )
            # bounce-in split across both HWDGE queues: the two transfers'
            # HBM-read completion receipts overlap (saves ~0.4us before the
            # first Sinkhorn op); partition bases 0/64 are 32-aligned.
            csum = sk.tile([K, N], f32)
            nc.sync.dma_start(out=csum[0:64, :], in_=dout[0:64, :])
            nc.scalar.dma_start(out=csum[64:K, :], in_=dout[64:K, :])

            # log_alpha = (dot - 1)/TEMP = csum - 1/TEMP up to an additive
            # constant which cancels in the first row logsumexp.
            cur = csum  # [K, N]
            for it in range(ITERS):
                # rows: lse over free dim of [K, N]
                nM = sk.tile([K, 1], f32)
                nc.vector.reduce_max(
                    out=nM, in_=cur, axis=mybir.AxisListType.X, negate=True
                )
                E = tps.tile([K, N], f32, tag="escr", bufs=1)
                S = sk.tile([K, 1], f32)
                nc.scalar.activation(out=E, in_=cur, func=Exp, bias=nM, accum_out=S)
                lS = sk.tile([K, 1], f32)
                nc.scalar.activation(out=lS, in_=S, func=Ln)
                L2 = sk.tile([K, N], f32)
                nc.vector.tensor_scalar(
                    out=L2,
                    in0=cur,
                    scalar1=nM,
                    scalar2=lS,
                    op0=mybir.AluOpType.add,
                    op1=mybir.AluOpType.subtract,
                )
                # cols: transpose, lse over free
                TpP = tps.tile([N, K], f32)
                nc.tensor.transpose(TpP, L2, ident[:K, :K])
                nM2 = sk.tile([N, 1], f32)
                nc.vector.reduce_max(
                    out=nM2, in_=TpP, axis=mybir.AxisListType.X, negate=True
                )
                E2 = tps.tile([N, K], f32, tag="escr2", bufs=1)
                S2 = sk.tile([N, 1], f32)
                nc.scalar.activation(out=E2, in_=TpP, func=Exp, bias=nM2, accum_out=S2)
                lS2 = sk.tile([N, 1], f32)
                nc.scalar.activation(out=lS2, in_=S2, func=Ln)
                if it < ITERS - 1:
                    Lt2 = sk.tile([N, K], f32)
                    nc.vector.tensor_scalar(
                        out=Lt2,
                        in0=TpP,
                        scalar1=nM2,
                        scalar2=lS2,
                        op0=mybir.AluOpType.add,
                        op1=mybir.AluOpType.subtract,
                    )
                    Tp2P = tps.tile([K, N], f32)
                    nc.tensor.transpose(Tp2P, Lt2, ident[:N, :N])
                    cur = Tp2P
                else:
                    # final col step: fuse normalize+exp into one activation
                    # res = exp(TpP + nM2 - lS2); host transposes [N,K]->[K,N]
                    bias_t = sk.tile([N, 1], f32)
                    nc.vector.tensor_scalar(
                        out=bias_t, in0=nM2, scalar1=lS2, scalar2=None,
                        op0=mybir.AluOpType.subtract,
                    )
                    res = sk.tile([N, K], f32)
                    nc.scalar.activation(out=res, in_=TpP, func=Exp, bias=bias_t)
                    # output split across both queues: completion receipts
                    # overlap at the kernel end
                    nc.sync.dma_start(out=out[0:32, :], in_=res[0:32, :])
                    nc.scalar.dma_start(out=out[32:N, :], in_=res[32:N, :])

    nc.compile()
    return nc


def _get_nc():
    if "nc" not in _CACHE:
        _CACHE["nc"] = _build()
    return _CACHE["nc"]


def _get_runner():
    """Cached PJRT executable (mirrors bass2jax.run_bass_via_pjrt's multi-core
    branch) so repeat kernel() calls skip retracing/recompiling."""
    if "runner" in _CACHE:
        return _CACHE["runner"]
    import jax
    from jax.experimental.shard_map import shard_map
    from jax.sharding import Mesh, PartitionSpec

    from concourse import bass2jax, mybir

    nc = _get_nc()
    bass2jax.install_neuronx_cc_hook()
    assert nc.dbg_addr is None
    partition_name = nc.partition_id_tensor.name if nc.partition_id_tensor else None

    in_names, out_names, out_avals, out_shapes = [], [], [], []
    for alloc in nc.m.functions[0].allocations:
        if not isinstance(alloc, mybir.MemoryLocationSet):
            continue
        name = alloc.memorylocations[0].name
        if alloc.kind == "ExternalInput":
            if name != partition_name:
                in_names.append(name)
        elif alloc.kind == "ExternalOutput":
            shape = tuple(alloc.tensor_shape)
            dtype = mybir.dt.np(alloc.dtype)
            out_avals.append(jax.core.ShapedArray(shape, dtype))
            out_shapes.append((name, shape, dtype))
            out_names.append(name)
    n_params = len(in_names)
    n_outs = len(out_names)
    all_in_names = list(in_names) + list(out_names)
    if partition_name is not None:
        all_in_names.append(partition_name)
    donate = tuple(range(n_params, n_params + n_outs))

    def _body(*args):
        operands = list(args)
        if partition_name is not None:
            operands.append(bass2jax.partition_id_tensor())
        outs = bass2jax._bass_exec_p.bind(
            *operands,
            out_avals=tuple(out_avals),
            in_names=tuple(all_in_names),
            out_names=tuple(out_names),
            lowering_input_output_aliases=(),
            sim_require_finite=True,
            sim_require_nnan=True,
            nc=nc,
        )
        return tuple(outs)

    devices = jax.devices()[:CORES]
    mesh = Mesh(np.asarray(devices), ("core",))
    in_specs = (PartitionSpec("core"),) * (n_params + n_outs)
    out_specs = (PartitionSpec("core"),) * n_outs
    sharded = jax.jit(
        shard_map(
            _body, mesh=mesh, in_specs=in_specs, out_specs=out_specs, check_rep=False
        ),
        donate_argnums=donate,
        keep_unused=True,
    )
    _CACHE["runner"] = (sharded, in_names, out_shapes)
    return _CACHE["runner"]


def _pack(arr, rows):
    # arr [rows, HW] fp32 -> fp16 packed per core: q-step q of core c covers
    # HW [c*SHARD + q*512, +512), SBUF partition p holds FB=4 consecutive
    # elements per (q, row).  First NBLKA*QBA q-steps go to the big blocks,
    # the rest to the small tail blocks.
    v = arr.reshape(rows, CORES, Q, P, FB).transpose(1, 2, 3, 0, 4)
    v = v.astype(np.float16)  # [CORES, Q, P, rows, FB]
    na = NBLKA * QBA
    va = v[:, :na].reshape(CORES, NBLKA, QBA, P, rows, FB).transpose(0, 1, 3, 2, 4, 5)
    vb = v[:, na:].reshape(CORES, NBLKB, QBB, P, rows, FB).transpose(0, 1, 3, 2, 4, 5)
    return (
        np.ascontiguousarray(va).reshape(CORES, NBLKA, P, QBA * rows * FB),
        np.ascontiguousarray(vb).reshape(CORES, NBLKB, P, QBB * rows * FB),
    )


def kernel(pred_masks, gt_masks):
    global LAST_RESULT
    from concourse import bass_utils

    pred = np.ascontiguousarray(np.asarray(pred_masks, dtype=np.float32)).reshape(
        K, HW
    )
    gt = np.ascontiguousarray(np.asarray(gt_masks, dtype=np.float32)).reshape(N, HW)
    pka, pkb = _pack(pred, K)
    gka, gkb = _pack(gt, N)
    in_maps = [
        {"p_in": pka[c], "g_in": gka[c], "p_in2": pkb[c], "g_in2": gkb[c]}
        for c in range(CORES)
    ]
    if TRACE:
        nc = _get_nc()
        res = bass_utils.run_bass_kernel_spmd(
            nc, in_maps, core_ids=list(range(CORES)), trace=TRACE, **TRACE_KW
        )
        LAST_RESULT = res
        o = np.asarray(res.results[0]["out"], dtype=np.float32)
        return np.ascontiguousarray(o.T).reshape(1, K, N)

    sharded, in_names, out_shapes = _get_runner()
    concat_in = [
        np.concatenate([in_maps[c][name] for c in range(CORES)], axis=0)
        for name in in_names
    ]
    concat_zeros = [
        np.zeros((CORES * shape[0], *shape[1:]), dtype) for _, shape, dtype in out_shapes
    ]
    out_arrs = sharded(*concat_in, *concat_zeros)
    out0 = np.asarray(out_arrs[0]).reshape(CORES, N, K)[0]
    return np.ascontiguousarray(out0.astype(np.float32).T).reshape(1, K, N)
